# revision 6
# baseline (speedup 1.0000x reference)
"""Concordance-index loss on Trainium2 (8 NeuronCores, Bass, SPMD).

Reference math over N=8192 samples (t = exp(event_time), d = event_indicator,
r = estimate), pairwise over ordered pairs (i, j):
    comp(i,j)  = d_i & (t_i < t_j | (t_i == t_j & ~d_j))
    conc       = sum comp & (r_j - r_i < 0)
    tied       = sum comp & |r_j - r_i| <= 1e-8
    total      = sum comp
    disc       = total - conc - tied
    out        = 1 - (disc + 0.5*tied) / (disc + conc + tied + 1e-7)

Decomposition (host does O(N log N) rank/sort prep; the dense O(N^2)
pairwise counting runs on the NeuronCores):

 - Only event samples can be the i of a comparable pair.  Sort the E events
   by t-rank; then comp(i,j) = [i < c_j] is a PREFIX of the sorted order,
   with c_j = #{events with trank < u_j}, u_j = trank_j + 0.5*(1-d_j)
   (host searchsorted).  total = sum_j c_j needs no device work.
 - r is replaced by its dense rank over unique values, embedded as monotone
   fp16 bit patterns (1024 + 2*rank).view(f16); per-j thresholds use odd
   patterns (1024 + 2*rank + 1) so device compares are strict and tie-free:
     conc_j = #{i < c_j : remb_i > tgt_j},  tgt = emb(2*rrank_j + 1)
 - The tie band |fl32(r_i - r_j)| <= 1e-8 is a contiguous r-rank window
   [lo_j, hi_j] (host, exact IEEE-f32 semantics).  For float survival data
   the bands are ~1 rank wide, so `tied` is O(N)-sparse: counted exactly on
   host via combined-key searchsorted over (rank, t-position).  If bands are
   wide (heavily quantized r), fall back to counting all three sums on
   device (variant v1).
 - j's are sorted by c_j into 64 batches of 128 (partition dim).  Slot s
   holds batches [8s, 8s+8); core c takes batch 8s+c, so ONE SPMD NEFF
   serves all cores with per-slot static shapes F_s = min c, C_s = max c
   over the slot: counts over the prefix [0, F_s) are UNGATED reductions
   (tensor_scalar compare + accum on DVE in its 4x perf mode, or the
   activation-Sign trick on the Activation engine — both run concurrently);
   only the window [F_s, C_s) needs the [i < c_j] gate (iota-row compare +
   scalar_tensor_tensor on DVE).  Pool/GPSIMD cannot execute ALU ops under
   neuronxcc, but its SWDGE queue still serves as a third parallel DMA
   channel: the fp16 remb halves go out on the SP + Act HWDGE queues and
   iota+scalars on the Pool queue, all partition-broadcast (stride-0) so
   the host ships each row once.
"""

import numpy as np

N = 8192
NCORES = 8
P = 128
NBATCH = N // P            # 64 j-batches of 128
NSLOTS = NBATCH // NCORES  # 8 slots; slot s = batches [8s, 8s+8)

DVE, ACT = "dve", "act"

_CACHE = {}


# ----------------------------------------------------------------- host prep

def _tie_windows(rv):
    """For each unique sorted f32 value rv[k], the contiguous index window
    [lo[k], hi[k]] with |fl32(rv[p] - rv[k])| <= 1e-8 (exact IEEE f32
    semantics, matching the reference's rdiff).  Vectorized: f64 prefilter
    with slack, then exact f32 checks on boundary candidates."""
    m = len(rv)
    thr = np.float32(1e-8)
    rv64 = rv.astype(np.float64)
    pad = np.float64(1.1e-8)
    lo_ap = np.searchsorted(rv64, rv64 - pad, side="left")
    hi_ap = np.searchsorted(rv64, rv64 + pad, side="right") - 1

    def exact_in(k_idx, p_idx):
        p_idx = np.clip(p_idx, 0, m - 1)
        d = rv[p_idx] - rv[k_idx]          # exact f32 subtract
        return np.abs(d) <= thr

    ks = np.arange(m)
    lo = np.full(m, -1, dtype=np.int64)
    for off in range(-2, 3):
        cand = np.clip(lo_ap + off, 0, m - 1)
        ok = exact_in(ks, cand) & (lo < 0)
        lo[ok] = cand[ok]
    hi = np.full(m, -1, dtype=np.int64)
    for off in range(2, -3, -1):
        cand = np.clip(hi_ap + off, 0, m - 1)
        ok = exact_in(ks, cand) & (hi < 0)
        hi[ok] = cand[ok]
    bad = (lo < 0) | (lo > ks)
    lo[bad] = ks[bad]
    bad = (hi < 0) | (hi < ks)
    hi[bad] = ks[bad]
    return lo, hi


def _prep(event_indicator, event_time, estimate):
    d = np.asarray(event_indicator).reshape(-1).astype(bool)
    t = np.asarray(event_time, dtype=np.float32).reshape(-1)
    r = np.asarray(estimate, dtype=np.float32).reshape(-1)
    n = t.shape[0]
    assert n == N

    # t dense ranks (exp is strictly monotone, so raw times rank identically)
    tv = np.unique(t)
    trk = np.searchsorted(tv, t).astype(np.int64)
    assert len(tv) < 2040, "t ranks must stay fp16-exact"
    u = trk.astype(np.float32) + np.float32(0.5) * (~d).astype(np.float32)

    # events sorted by t-rank
    ev = np.nonzero(d)[0]
    E = len(ev)
    order = np.argsort(trk[ev], kind="stable")
    eidx = ev[order]
    ttr = trk[eidx]                       # sorted event t-ranks
    E_pad = max(((E + P - 1) // P) * P, P)

    # r dense ranks -> strict/odd monotone fp16 bit-pattern embedding
    rv = np.unique(r)
    m = len(rv)
    assert 1024 + 2 * m < 31744, "r rank embedding must stay normal fp16"
    rrk = np.searchsorted(rv, r)
    lo, hi = _tie_windows(rv)

    def emb(v):
        return (1024 + v).astype(np.uint16).view(np.float16)

    c = np.searchsorted(ttr, u, side="left").astype(np.int64)  # prefix cut
    total = float(c.sum())

    jorder = np.argsort(c, kind="stable")
    csort = c[jorder]

    slots = []
    for s in range(NSLOTS):
        cs = csort[s * NCORES * P:(s + 1) * NCORES * P]
        F = int(cs[0])
        C = int(cs[-1])
        slots.append({"F": F, "C": C, "W": C - F})

    return dict(d=d, u=u, trk=trk, eidx=eidx, ttr=ttr, E=E, E_pad=E_pad,
                rrk=rrk, lo=lo, hi=hi, emb=emb, c=c, total=total,
                jorder=jorder, slots=slots)


def _tied_on_host(pp):
    """tied = sum_j #{events i : i < c_j, rank_i in [lo_j, hi_j]} via
    combined-key searchsorted (exact).  Cost O(N log N + band width)."""
    rrk, lo, hi, c = pp["rrk"], pp["lo"], pp["hi"], pp["c"]
    eidx = pp["eidx"]
    BIG = np.int64(1 << 20)
    keys = np.sort(rrk[eidx].astype(np.int64) * BIG
                   + np.arange(len(eidx), dtype=np.int64))
    jr = rrk.astype(np.int64)
    jlo, jhi = lo[jr], hi[jr]

    def band_cnt(q, cj):                     # events rank==q with pos < c_j
        a = np.searchsorted(keys, q * BIG, side="left")
        b = np.searchsorted(keys, q * BIG + cj, side="left")
        return b - a

    tied = band_cnt(jr, c).sum()             # own-rank part, all j
    wide = np.nonzero(jhi > jlo)[0]
    for j in wide:
        for q in range(int(jlo[j]), int(jhi[j]) + 1):
            if q == jr[j]:
                continue
            tied += band_cnt(np.int64(q), int(c[j]))
    return float(tied)


# ----------------------------------------------------------- engine planning

def _seg_cost(eng, L):
    if eng == DVE:
        return 0.26 * L + 110
    return 0.833 * L + 420          # ACT


def _plan_v2(slots):
    """DVE: all gated windows (comp + stt) + a share of full prefixes;
    ACT: the remaining full prefixes (Sign trick).  Full prefixes may be
    split into two segments across the engines."""
    loads = {DVE: 0.0, ACT: 0.0}
    items = []
    for s in range(NSLOTS):
        W = slots[s]["W"]
        if W == 0:
            continue
        loads[DVE] += 1.30 * W + 220
        items.append({"kind": "gated", "slot": s, "eng": DVE})

    segs = []
    for s in sorted(range(NSLOTS), key=lambda s: -slots[s]["F"]):
        F = slots[s]["F"]
        if F == 0:
            continue
        eng = min((DVE, ACT), key=lambda e: loads[e] + _seg_cost(e, F))
        loads[eng] += _seg_cost(eng, F)
        segs.append({"kind": "full", "slot": s, "eng": eng, "seg": (0, F)})
    # refinement: move a chunk of the heaviest engine's largest seg
    for _ in range(16):
        hi_e = max(loads, key=loads.get)
        lo_e = min(loads, key=loads.get)
        cand = [g for g in segs if g["eng"] == hi_e and g["seg"][1] >= 512]
        if not cand or loads[hi_e] - loads[lo_e] < 600:
            break
        g = max(cand, key=lambda g: g["seg"][1])
        st, L = g["seg"]
        r_hi = 0.26 if hi_e == DVE else 0.833
        r_lo = 0.26 if lo_e == DVE else 0.833
        ovh = 110 if lo_e == DVE else 420
        gap = loads[hi_e] - loads[lo_e]
        move = int(min(L - 256, max(0, (gap - ovh) / (r_hi + r_lo))))
        if move < 256:
            break
        g["seg"] = (st, L - move)
        loads[hi_e] -= r_hi * move
        loads[lo_e] += r_lo * move + ovh
        segs.append({"kind": "full", "slot": g["slot"], "eng": lo_e,
                     "seg": (st + L - move, move)})
    items.extend(segs)
    for i, it in enumerate(items):
        it["col"] = i
    return items, max(len(items), 1)


def _plan_v1(slots):
    """Three counts on device (fallback for wide tie bands)."""
    loads = {DVE: 0.0, ACT: 0.0}
    items = []
    for s in range(NSLOTS):
        W = slots[s]["W"]
        if W == 0:
            continue
        loads[DVE] += 0.26 * W + 110 + 3 * (1.04 * W + 110)
        for k in range(3):
            items.append({"kind": "gated", "slot": s, "count": k,
                          "eng": DVE})
    for s in sorted(range(NSLOTS), key=lambda s: -slots[s]["F"]):
        F = slots[s]["F"]
        if F == 0:
            continue
        for k in range(3):
            eng = min((DVE, ACT), key=lambda e: loads[e] + _seg_cost(e, F))
            loads[eng] += _seg_cost(eng, F)
            items.append({"kind": "full", "slot": s, "count": k, "eng": eng,
                          "seg": (0, F)})
    for i, it in enumerate(items):
        it["col"] = i
    return items, max(len(items), 1)


# ------------------------------------------------------------------ nc build

def _build_nc(slots, E_pad, Wmax, items, ncols, nsc):
    """Shared builder.  nsc = scalar columns per slot (2 for v2: c-F, tgt;
    4 for v1: u-F..., tgt, tlo, thi — gated compares use iota vs (c-F) or
    window-local u)."""
    import concourse.bass as bass
    from concourse import mybir

    dt = mybir.dt
    Alu = mybir.AluOpType
    Af = mybir.ActivationFunctionType

    nc = bass.Bass()
    xrows = nc.declare_dram_parameter("xrows", [1, E_pad + Wmax], dt.float16,
                                      isOutput=False)
    xsc = nc.declare_dram_parameter("xsc", [P, nsc * NSLOTS], dt.float32,
                                    isOutput=False)
    out = nc.declare_dram_parameter("out", [P, ncols], dt.float32,
                                    isOutput=True)
    HALF = (E_pad // 2 // P) * P

    with (
        nc.sbuf_tensor([P, E_pad], dt.float16) as remb,
        nc.sbuf_tensor([P, Wmax], dt.float16) as iota,
        nc.sbuf_tensor([P, nsc * NSLOTS], dt.float32) as sc,
        nc.sbuf_tensor([P, ncols], dt.float32) as acc,
        nc.sbuf_tensor([P, E_pad], dt.float16) as dead_v,
        nc.sbuf_tensor([P, E_pad], dt.float16) as dead_a,
        nc.sbuf_tensor([P, Wmax], dt.float16) as comp_v,
        nc.semaphore() as d1,
        nc.semaphore() as d2,
        nc.semaphore() as d3,
        nc.semaphore() as vsem,
        nc.Block() as block,
    ):
        def cutcol(s):
            return sc[:, nsc * s + 0:nsc * s + 1]

        def thrcol(s, k):
            return sc[:, nsc * s + 1 + k:nsc * s + 2 + k]

        @block.sync
        def _(sp):
            sp.dma_start(remb[:, 0:HALF], xrows[0:1, 0:HALF]
                         .partition_broadcast(P)).then_inc(d1, 16)
            sp.wait_ge(vsem, 2)
            sp.dma_start(out[:], acc[:]).then_inc(d1, 16)

        @block.gpsimd
        def _(g):
            g.dma_start(iota[:], xrows[0:1, E_pad:E_pad + Wmax]
                        .partition_broadcast(P)).then_inc(d3, 16)
            g.dma_start(sc[:], xsc[:]).then_inc(d3, 16)

        @block.scalar
        def _(a):
            a.dma_start(remb[:, HALF:E_pad], xrows[0:1, HALF:E_pad]
                        .partition_broadcast(P)).then_inc(d2, 16)
            a.wait_ge(d1, 16)
            a.wait_ge(d2, 16)
            a.wait_ge(d3, 32)
            last = None
            for it in items:
                if it["eng"] != ACT:
                    continue
                st, L = it["seg"]
                last = a.activation(
                    dead_a[:, 0:L], remb[:, st:st + L], Af.Sign,
                    bias=thrcol(it["slot"], it.get("count", 0)), scale=-1.0,
                    accum_out=acc[:, it["col"]:it["col"] + 1])
            if last is None:
                last = a.activation(dead_a[:, 0:1], remb[:, 0:1], Af.Sign,
                                    bias=0.0, scale=1.0)
            last.then_inc(vsem, 1)

        @block.vector
        def _(v):
            v.wait_ge(d1, 16)
            v.wait_ge(d2, 16)
            v.wait_ge(d3, 32)
            last = None
            for s in range(NSLOTS):
                gated = [it for it in items
                         if it["eng"] == DVE and it["kind"] == "gated"
                         and it["slot"] == s]
                if not gated:
                    continue
                F, W = slots[s]["F"], slots[s]["W"]
                v.tensor_scalar(comp_v[:, 0:W], iota[:, 0:W], cutcol(s),
                                None, Alu.is_lt)
                for it in gated:
                    last = v.scalar_tensor_tensor(
                        dead_v[:, 0:W], remb[:, F:F + W],
                        thrcol(s, it.get("count", 0)), comp_v[:, 0:W],
                        op0=Alu.is_gt, op1=Alu.mult,
                        accum_out=acc[:, it["col"]:it["col"] + 1])
            for it in items:
                if it["eng"] != DVE or it["kind"] != "full":
                    continue
                st, L = it["seg"]
                last = v.tensor_scalar(
                    dead_v[:, 0:L], remb[:, st:st + L],
                    thrcol(it["slot"], it.get("count", 0)), None, Alu.is_gt,
                    op1=Alu.add, accum_out=acc[:, it["col"]:it["col"] + 1])
            if last is None:
                last = v.memset(dead_v[:, 0:1], 0.0)
            last.then_inc(vsem, 1)

    return nc


# --------------------------------------------------------------- input maps

def _inputs(pp, Wmax, nsc):
    emb, rrk, eidx, c, jorder = (pp["emb"], pp["rrk"], pp["eidx"], pp["c"],
                                 pp["jorder"])
    E, E_pad, slots = pp["E"], pp["E_pad"], pp["slots"]
    remb_row = np.zeros(E_pad, dtype=np.float16)
    remb_row[:E] = emb(2 * rrk[eidx])
    iota_row = np.arange(Wmax, dtype=np.float16)
    rows = np.concatenate([remb_row, iota_row])[None, :]
    tgt = emb(2 * rrk + 1).astype(np.float32)
    if nsc == 4:
        tlo = emb(2 * pp["lo"][rrk] - 1).astype(np.float32)
        thi = emb(2 * pp["hi"][rrk] + 1).astype(np.float32)

    sc = np.zeros((NCORES, P, nsc * NSLOTS), dtype=np.float32)
    for s in range(NSLOTS):
        F = slots[s]["F"]
        for cr in range(NCORES):
            b = s * NCORES + cr
            jj = jorder[b * P:(b + 1) * P]
            sc[cr, :, nsc * s + 0] = (c[jj] - F).astype(np.float32)
            sc[cr, :, nsc * s + 1] = tgt[jj]
            if nsc == 4:
                sc[cr, :, nsc * s + 2] = tlo[jj]
                sc[cr, :, nsc * s + 3] = thi[jj]
    return [{"xrows": np.ascontiguousarray(rows),
             "xsc": np.ascontiguousarray(sc[cr])} for cr in range(NCORES)]


# ----------------------------------------------------------------- finishing

def _counts(results, items, nsums):
    acc = [np.float64(0.0)] * nsums
    for res in results:
        o = res["out"].astype(np.float64)
        for it in items:
            col = o[:, it["col"]]
            if it["kind"] == "full" and it["eng"] == ACT:
                L = it["seg"][1]
                cnt = (P * L - col.sum()) / 2.0
            else:
                cnt = col.sum()
            acc[it.get("count", 0)] += cnt
    return acc


def _formula(total, conc, tied):
    disc = total - conc - tied
    loss = (disc + 0.5 * tied) / (disc + conc + tied + 1e-7)
    return np.asarray(1.0 - loss, dtype=np.float32)


# --------------------------------------------------------------- entry point

def kernel(event_indicator, event_time, estimate):
    from concourse.bass_utils import run_bass_kernel_spmd

    pp = _prep(event_indicator, event_time, estimate)
    slots, E_pad = pp["slots"], pp["E_pad"]
    total = pp["total"]
    if total == 0.0:
        return np.float32(1.0)

    jr = pp["rrk"]
    band = (pp["hi"][jr] - pp["lo"][jr]).sum()
    Wmax = max(max(s["W"] for s in slots), 1)
    use_v2 = band <= 8 * N

    if use_v2:
        tied = _tied_on_host(pp)
        items, ncols = _plan_v2(slots)
        nsc = 2
    else:
        tied = None
        items, ncols = _plan_v1(slots)
        nsc = 4

    key = (use_v2, E_pad, Wmax, ncols,
           tuple((s["F"], s["C"]) for s in slots),
           tuple((it["kind"], it["slot"], it.get("count", 0), it["eng"],
                  it.get("seg")) for it in items))
    if _CACHE.get("key") != key:
        _CACHE["nc"] = _build_nc(slots, E_pad, Wmax, items, ncols, nsc)
        _CACHE["key"] = key
    in_maps = _inputs(pp, Wmax, nsc)
    out = run_bass_kernel_spmd(_CACHE["nc"], in_maps,
                               core_ids=list(range(NCORES)))
    if use_v2:
        (conc,) = _counts(out.results, items, 1)
        return _formula(total, conc, tied)
    conc, cntA, cntB = _counts(out.results, items, 3)
    return _formula(total, conc, cntA - cntB)


# revision 22
# speedup vs baseline: 1.0598x; 1.0598x over previous
"""Concordance-index loss on Trainium2 (8 NeuronCores, Bass, SPMD).

Reference math over N=8192 samples (t = exp(event_time), d = event_indicator,
r = estimate), pairwise over ordered pairs (i, j):
    comp(i,j)  = d_i & (t_i < t_j | (t_i == t_j & ~d_j))
    conc       = sum comp & (r_j - r_i < 0)
    tied       = sum comp & |r_j - r_i| <= 1e-8
    total      = sum comp
    disc       = total - conc - tied
    out        = 1 - (disc + 0.5*tied) / (disc + conc + tied + 1e-7)

Decomposition (host does O(N log N) rank/sort prep; the dense O(N^2)
pairwise counting runs on the NeuronCores):

 - Only event samples can be the i of a comparable pair.  Sort the E events
   by t-rank; then comp(i,j) = [i < c_j] is a PREFIX of the sorted order,
   with c_j = #{events with trank < u_j}, u_j = trank_j + 0.5*(1-d_j)
   (host searchsorted).  total = sum_j c_j needs no device work.
 - r is replaced by its dense rank over unique values, embedded as monotone
   fp16 bit patterns (1024 + 2*rank).view(f16); per-j thresholds use odd
   patterns (1024 + 2*rank + 1) so device compares are strict and tie-free:
     conc_j = #{i < c_j : remb_i > tgt_j},  tgt = emb(2*rrank_j + 1)
 - The tie band |fl32(r_i - r_j)| <= 1e-8 is a contiguous r-rank window
   [lo_j, hi_j] (host, exact IEEE-f32 semantics).  For float survival data
   the bands are ~1 rank wide, so `tied` is O(N)-sparse: counted exactly on
   host via combined-key searchsorted over (rank, t-position).  If bands are
   wide (heavily quantized r), fall back to counting all three sums on
   device (variant v1).
 - j's are sorted by c_j into 64 batches of 128 (partition dim).  Slot s
   holds batches [8s, 8s+8); core c takes batch 8s+c, so ONE SPMD NEFF
   serves all cores with per-slot static shapes F_s = min c, C_s = max c
   over the slot: counts over the prefix [0, F_s) are UNGATED reductions
   (tensor_scalar compare + accum on DVE in its 4x perf mode, or the
   activation-Sign trick on the Activation engine — both run concurrently);
   only the window [F_s, C_s) needs the [i < c_j] gate (iota-row compare +
   scalar_tensor_tensor on DVE).  Pool/GPSIMD cannot execute ALU ops under
   neuronxcc, but its SWDGE queue still serves as a third parallel DMA
   channel: the fp16 remb halves go out on the SP + Act HWDGE queues and
   iota+scalars on the Pool queue, all partition-broadcast (stride-0) so
   the host ships each row once.
"""

import numpy as np

N = 8192
NCORES = 8
P = 128
NBATCH = N // P            # 64 j-batches of 128
NSLOTS = NBATCH // NCORES  # 8 slots; slot s = batches [8s, 8s+8)

DVE, ACT = "dve", "act"

_CACHE = {}


# ----------------------------------------------------------------- host prep

def _tie_windows(rv):
    """For each unique sorted f32 value rv[k], the contiguous index window
    [lo[k], hi[k]] with |fl32(rv[p] - rv[k])| <= 1e-8 (exact IEEE f32
    semantics, matching the reference's rdiff).  Vectorized: f64 prefilter
    with slack, then exact f32 checks on boundary candidates."""
    m = len(rv)
    thr = np.float32(1e-8)
    rv64 = rv.astype(np.float64)
    pad = np.float64(1.1e-8)
    lo_ap = np.searchsorted(rv64, rv64 - pad, side="left")
    hi_ap = np.searchsorted(rv64, rv64 + pad, side="right") - 1

    def exact_in(k_idx, p_idx):
        p_idx = np.clip(p_idx, 0, m - 1)
        d = rv[p_idx] - rv[k_idx]          # exact f32 subtract
        return np.abs(d) <= thr

    ks = np.arange(m)
    lo = np.full(m, -1, dtype=np.int64)
    for off in range(-2, 3):
        cand = np.clip(lo_ap + off, 0, m - 1)
        ok = exact_in(ks, cand) & (lo < 0)
        lo[ok] = cand[ok]
    hi = np.full(m, -1, dtype=np.int64)
    for off in range(2, -3, -1):
        cand = np.clip(hi_ap + off, 0, m - 1)
        ok = exact_in(ks, cand) & (hi < 0)
        hi[ok] = cand[ok]
    bad = (lo < 0) | (lo > ks)
    lo[bad] = ks[bad]
    bad = (hi < 0) | (hi < ks)
    hi[bad] = ks[bad]
    return lo, hi


def _prep(event_indicator, event_time, estimate):
    d = np.asarray(event_indicator).reshape(-1).astype(bool)
    t = np.asarray(event_time, dtype=np.float32).reshape(-1)
    r = np.asarray(estimate, dtype=np.float32).reshape(-1)
    n = t.shape[0]
    assert n == N

    # t dense ranks (exp is strictly monotone, so raw times rank identically)
    tv = np.unique(t)
    trk = np.searchsorted(tv, t).astype(np.int64)
    assert len(tv) < 2040, "t ranks must stay fp16-exact"
    u = trk.astype(np.float32) + np.float32(0.5) * (~d).astype(np.float32)

    # events sorted by t-rank
    ev = np.nonzero(d)[0]
    E = len(ev)
    order = np.argsort(trk[ev], kind="stable")
    eidx = ev[order]
    ttr = trk[eidx]                       # sorted event t-ranks
    E_pad = max(((E + P - 1) // P) * P, P)

    # r dense ranks -> strict/odd monotone fp16 bit-pattern embedding
    rv = np.unique(r)
    m = len(rv)
    assert 1024 + 2 * m < 31744, "r rank embedding must stay normal fp16"
    rrk = np.searchsorted(rv, r)
    lo, hi = _tie_windows(rv)

    def emb(v):
        return (1024 + v).astype(np.uint16).view(np.float16)

    c = np.searchsorted(ttr, u, side="left").astype(np.int64)  # prefix cut
    total = float(c.sum())

    jorder = np.argsort(c, kind="stable")
    csort = c[jorder]

    slots = []
    for s in range(NSLOTS):
        cs = csort[s * NCORES * P:(s + 1) * NCORES * P]
        F = int(cs[0])
        C = int(cs[-1])
        slots.append({"F": F, "C": C, "W": C - F})

    return dict(d=d, u=u, trk=trk, eidx=eidx, ttr=ttr, E=E, E_pad=E_pad,
                rrk=rrk, lo=lo, hi=hi, emb=emb, c=c, total=total,
                jorder=jorder, slots=slots)


def _tied_on_host(pp):
    """tied = sum_j #{events i : i < c_j, rank_i in [lo_j, hi_j]} via
    combined-key searchsorted (exact).  Cost O(N log N + band width)."""
    rrk, lo, hi, c = pp["rrk"], pp["lo"], pp["hi"], pp["c"]
    eidx = pp["eidx"]
    BIG = np.int64(1 << 20)
    keys = np.sort(rrk[eidx].astype(np.int64) * BIG
                   + np.arange(len(eidx), dtype=np.int64))
    jr = rrk.astype(np.int64)
    jlo, jhi = lo[jr], hi[jr]

    def band_cnt(q, cj):                     # events rank==q with pos < c_j
        a = np.searchsorted(keys, q * BIG, side="left")
        b = np.searchsorted(keys, q * BIG + cj, side="left")
        return b - a

    tied = band_cnt(jr, c).sum()             # own-rank part, all j
    wide = np.nonzero(jhi > jlo)[0]
    for j in wide:
        for q in range(int(jlo[j]), int(jhi[j]) + 1):
            if q == jr[j]:
                continue
            tied += band_cnt(np.int64(q), int(c[j]))
    return float(tied)


# ----------------------------------------------------------- engine planning

def _seg_cost(eng, L):
    if eng == DVE:
        return 0.26 * L + 90
    return 0.833 * L + 430          # ACT


def _plan_v2(slots, E_pad):
    """DVE: all gated windows (comp + stt) + a share of full prefixes;
    ACT: the remaining full prefixes (Sign trick).  Full prefixes are
    pre-split at the stage boundary MID (end of the second DMA chunk) so
    stage-A work can start before the whole remb row lands."""
    MID = 2 * (E_pad // 4)
    loads = {DVE: 0.0, ACT: 150.0}
    items = []
    for s in range(NSLOTS):
        W = slots[s]["W"]
        if W == 0:
            continue
        loads[DVE] += 1.34 * W + 150
        items.append({"kind": "gated", "slot": s, "eng": DVE, "counts": [0],
                      "ext": slots[s]["C"]})

    pieces = []
    for s in range(NSLOTS):
        F = slots[s]["F"]
        if F == 0:
            continue
        pieces.append((s, 0, 0, min(F, MID)))
        if F > MID:
            pieces.append((s, 0, MID, F - MID))
    segs = _assign_full(pieces, loads, MID)
    items.extend(segs)
    return _assign_cols(items)


def _plan_v1(slots, E_pad):
    """Three counts on device (fallback for wide tie bands)."""
    MID = 2 * (E_pad // 4)
    loads = {DVE: 0.0, ACT: 150.0}
    items = []
    for s in range(NSLOTS):
        W = slots[s]["W"]
        if W == 0:
            continue
        loads[DVE] += 0.26 * W + 90 + 3 * (1.04 * W + 90)
        items.append({"kind": "gated", "slot": s, "eng": DVE,
                      "counts": [0, 1, 2], "ext": slots[s]["C"]})
    pieces = []
    for s in range(NSLOTS):
        F = slots[s]["F"]
        if F == 0:
            continue
        for k in range(3):
            pieces.append((s, k, 0, min(F, MID)))
            if F > MID:
                pieces.append((s, k, MID, F - MID))
    segs = _assign_full(pieces, loads, MID)
    items.extend(segs)
    return _assign_cols(items)


def _assign_full(pieces, loads, MID):
    """Greedy engine assignment of full-count pieces (slot, count, st, L),
    then one split-refinement toward balance."""
    segs = []
    for (s, k, st, L) in sorted(pieces, key=lambda p: -p[3]):
        eng = min((DVE, ACT), key=lambda e: loads[e] + _seg_cost(e, L))
        loads[eng] += _seg_cost(eng, L)
        segs.append({"kind": "full", "slot": s, "count": k, "eng": eng,
                     "seg": (st, L), "ext": st + L})
    for _ in range(16):
        hi_e = max(loads, key=loads.get)
        lo_e = min(loads, key=loads.get)
        cand = [g for g in segs if g["eng"] == hi_e and g["seg"][1] >= 512]
        if not cand or loads[hi_e] - loads[lo_e] < 300:
            break
        g = max(cand, key=lambda g: g["seg"][1])
        st, L = g["seg"]
        r_hi = 0.26 if hi_e == DVE else 0.833
        r_lo = 0.26 if lo_e == DVE else 0.833
        ovh = 90 if lo_e == DVE else 430
        gap = loads[hi_e] - loads[lo_e]
        move = int(min(L - 256, max(0, (gap - ovh) / (r_hi + r_lo))))
        if move < 256:
            break
        g["seg"] = (st, L - move)
        loads[hi_e] -= r_hi * move
        loads[lo_e] += r_lo * move + ovh
        segs.append({"kind": "full", "slot": g["slot"], "count": g["count"],
                     "eng": lo_e, "seg": (st + L - move, move),
                     "ext": st + L})
    return segs


def _assign_cols(items):
    cur = 0
    for it in items:
        it["col"] = cur
        cur += len(it["counts"]) if it["kind"] == "gated" else 1
    return items, max(cur, 1)


# ------------------------------------------------------------------ nc build

def _build_nc(slots, E_pad, Wmax, items, ncols, nsc):
    """Shared builder.  nsc = scalar columns per slot (2 for v2, 4 for v1).
    DMA: two HWDGE queues only (SP + Act); scalars and the iota row go
    first (tiny), then the remb row in four stride-0 partition-broadcast
    chunks alternating between the queues.  Stage-A items need only
    remb[0:MID]; stage-B the full row."""
    import concourse.bass as bass
    from concourse import mybir

    dt = mybir.dt
    Alu = mybir.AluOpType
    Af = mybir.ActivationFunctionType

    nc = bass.Bass()
    xrows = nc.declare_dram_parameter("xrows", [1, E_pad], dt.float16,
                                      isOutput=False)
    xsc = nc.declare_dram_parameter("xsc", [P, nsc * NSLOTS], dt.float32,
                                    isOutput=False)
    out = nc.declare_dram_parameter("out", [P, ncols], dt.float32,
                                    isOutput=True)
    CH = E_pad // 4
    MID = 2 * CH

    iota_dt = dt.float16 if Wmax <= 2040 else dt.float32
    with (
        nc.sbuf_tensor([P, E_pad], dt.float16) as remb,
        nc.sbuf_tensor([P, Wmax], iota_dt) as iota,
        nc.sbuf_tensor([P, nsc * NSLOTS], dt.float32) as sc,
        nc.sbuf_tensor([P, ncols], dt.float32) as acc,
        nc.sbuf_tensor([P, E_pad], dt.float16) as dead_v,
        nc.sbuf_tensor([P, E_pad], dt.float16) as dead_a,
        nc.sbuf_tensor([P, Wmax], dt.float16) as comp_v,
        nc.semaphore() as d1,
        nc.semaphore() as d2,
        nc.semaphore() as psem,
        nc.semaphore() as vsem,
        nc.Block() as block,
    ):
        def cutcol(s):
            return sc[:, nsc * s + 0:nsc * s + 1]

        def thrcol(s, k):
            return sc[:, nsc * s + 1 + k:nsc * s + 2 + k]

        def rchunk(lo_, hi_):
            return (remb[:, lo_:hi_],
                    xrows[0:1, lo_:hi_].partition_broadcast(P))

        @block.sync
        def _(sp):
            sp.dma_start(sc[:], xsc[:]).then_inc(d1, 16)
            sp.dma_start(*rchunk(0, CH)).then_inc(d1, 16)
            sp.dma_start(*rchunk(CH, MID)).then_inc(d1, 16)
            sp.wait_ge(vsem, 2)
            sp.dma_start(out[:], acc[:]).then_inc(d1, 16)

        @block.gpsimd
        def _(g):
            g.dma_start(*rchunk(MID, 3 * CH)).then_inc(d2, 16)
            g.dma_start(*rchunk(3 * CH, E_pad)).then_inc(d2, 16)

        # wait ladder: SP queue (d1): scalars(16), chunk1(32), chunk2(48);
        # Pool queue (d2): chunk3(16), chunk4(32).  The iota row is built
        # on the DVE during the startup window (memset + scan), so gated
        # items need no extra wait.
        def level_of(it):
            ext = it["ext"]
            if ext <= CH:
                return (32, 0)
            if ext <= MID:
                return (48, 0)
            if ext <= 3 * CH:
                return (48, 16)
            return (48, 32)

        def emit_staged(eng_api, eng_name):
            last = None
            done = (0, 0)
            order = sorted((it for it in items if it["eng"] == eng_name),
                           key=level_of)
            for it in order:
                lv = level_of(it)
                if lv[0] > done[0]:
                    eng_api.wait_ge(d1, lv[0])
                if lv[1] > done[1]:
                    eng_api.wait_ge(d2, lv[1])
                done = (max(lv[0], done[0]), max(lv[1], done[1]))
                s = it["slot"]
                if it["kind"] == "gated":
                    F, W = slots[s]["F"], slots[s]["W"]
                    eng_api.tensor_scalar(
                        comp_v[:, 0:W], iota[:, 0:W], cutcol(s),
                        None, Alu.is_lt)
                    for k in it["counts"]:
                        last = eng_api.scalar_tensor_tensor(
                            dead_v[:, 0:W], remb[:, F:F + W],
                            thrcol(s, k), comp_v[:, 0:W],
                            op0=Alu.is_gt, op1=Alu.mult,
                            accum_out=acc[:, it["col"] + k:
                                          it["col"] + k + 1])
                else:
                    st, L = it["seg"]
                    k = it["count"]
                    if eng_name == DVE:
                        last = eng_api.tensor_scalar(
                            dead_v[:, 0:L], remb[:, st:st + L],
                            thrcol(s, k), None, Alu.is_gt,
                            op1=Alu.add,
                            accum_out=acc[:, it["col"]:it["col"] + 1])
                    else:
                        last = eng_api.activation(
                            dead_a[:, 0:L], remb[:, st:st + L], Af.Sign,
                            bias=thrcol(s, k), scale=-1.0,
                            accum_out=acc[:, it["col"]:it["col"] + 1])
            return last

        @block.scalar
        def _(a):
            # warm the Sign activation table while the DMAs are in flight
            a.wait_ge(psem, 1)
            a.activation(dead_a[:, 0:1], dead_a[:, 0:1], Af.Sign,
                         bias=0.0, scale=1.0)
            last = emit_staged(a, ACT)
            if last is None:
                a.wait_ge(d1, 32)
                a.wait_ge(d2, 32)
                last = a.activation(dead_a[:, 0:1], remb[:, 0:1], Af.Sign,
                                    bias=0.0, scale=1.0)
            last.then_inc(vsem, 1)

        @block.vector
        def _(v):
            v.memset(dead_a[:, 0:1], 0.0).then_inc(psem, 1)
            v.memset(comp_v[:], 1.0)
            v.tensor_tensor_scan(iota[:], comp_v[:], comp_v[:], -1.0,
                                 Alu.add, Alu.bypass)
            last = emit_staged(v, DVE)
            if last is None:
                v.wait_ge(d1, 32)
                v.wait_ge(d2, 32)
                last = v.memset(dead_v[:, 0:1], 0.0)
            last.then_inc(vsem, 1)

    return nc


# --------------------------------------------------------------- input maps

def _inputs(pp, Wmax, nsc):
    emb, rrk, eidx, c, jorder = (pp["emb"], pp["rrk"], pp["eidx"], pp["c"],
                                 pp["jorder"])
    E, E_pad, slots = pp["E"], pp["E_pad"], pp["slots"]
    remb_row = np.zeros(E_pad, dtype=np.float16)
    remb_row[:E] = emb(2 * rrk[eidx])
    rows = remb_row[None, :]
    tgt = emb(2 * rrk + 1).astype(np.float32)
    if nsc == 4:
        tlo = emb(2 * pp["lo"][rrk] - 1).astype(np.float32)
        thi = emb(2 * pp["hi"][rrk] + 1).astype(np.float32)

    sc = np.zeros((NCORES, P, nsc * NSLOTS), dtype=np.float32)
    for s in range(NSLOTS):
        F = slots[s]["F"]
        for cr in range(NCORES):
            b = s * NCORES + cr
            jj = jorder[b * P:(b + 1) * P]
            sc[cr, :, nsc * s + 0] = (c[jj] - F).astype(np.float32)
            sc[cr, :, nsc * s + 1] = tgt[jj]
            if nsc == 4:
                sc[cr, :, nsc * s + 2] = tlo[jj]
                sc[cr, :, nsc * s + 3] = thi[jj]
    return [{"xrows": np.ascontiguousarray(rows),
             "xsc": np.ascontiguousarray(sc[cr])} for cr in range(NCORES)]


# ----------------------------------------------------------------- finishing

def _counts(results, items, nsums):
    acc = [np.float64(0.0)] * nsums
    for res in results:
        o = res["out"].astype(np.float64)
        for it in items:
            if it["kind"] == "gated":
                for k in it["counts"]:
                    acc[k] += o[:, it["col"] + k].sum()
            elif it["eng"] == ACT:
                L = it["seg"][1]
                acc[it.get("count", 0)] += (P * L - o[:, it["col"]].sum()) / 2.0
            else:
                acc[it.get("count", 0)] += o[:, it["col"]].sum()
    return acc


def _formula(total, conc, tied):
    disc = total - conc - tied
    loss = (disc + 0.5 * tied) / (disc + conc + tied + 1e-7)
    return np.asarray(1.0 - loss, dtype=np.float32)


# --------------------------------------------------------------- entry point

def kernel(event_indicator, event_time, estimate):
    from concourse.bass_utils import run_bass_kernel_spmd

    pp = _prep(event_indicator, event_time, estimate)
    slots, E_pad = pp["slots"], pp["E_pad"]
    total = pp["total"]
    if total == 0.0:
        return np.float32(1.0)

    jr = pp["rrk"]
    band = (pp["hi"][jr] - pp["lo"][jr]).sum()
    Wmax = max(max(s["W"] for s in slots), 1)
    use_v2 = band <= 8 * N

    if use_v2:
        tied = _tied_on_host(pp)
        items, ncols = _plan_v2(slots, E_pad)
        nsc = 2
    else:
        tied = None
        items, ncols = _plan_v1(slots, E_pad)
        nsc = 4

    key = (use_v2, E_pad, Wmax, ncols,
           tuple((s["F"], s["C"]) for s in slots),
           tuple((it["kind"], it["slot"], it.get("count", 0), it["eng"],
                  it.get("seg")) for it in items))
    if _CACHE.get("key") != key:
        _CACHE["nc"] = _build_nc(slots, E_pad, Wmax, items, ncols, nsc)
        _CACHE["key"] = key
    in_maps = _inputs(pp, Wmax, nsc)
    out = run_bass_kernel_spmd(_CACHE["nc"], in_maps,
                               core_ids=list(range(NCORES)))
    if use_v2:
        (conc,) = _counts(out.results, items, 1)
        return _formula(total, conc, tied)
    conc, cntA, cntB = _counts(out.results, items, 3)
    return _formula(total, conc, cntA - cntB)


# revision 28
# speedup vs baseline: 1.0791x; 1.0182x over previous
"""Concordance-index loss on Trainium2 (8 NeuronCores, Bass, SPMD).

Reference math over N=8192 samples (t = exp(event_time), d = event_indicator,
r = estimate), pairwise over ordered pairs (i, j):
    comp(i,j)  = d_i & (t_i < t_j | (t_i == t_j & ~d_j))
    conc       = sum comp & (r_j - r_i < 0)
    tied       = sum comp & |r_j - r_i| <= 1e-8
    total      = sum comp
    disc       = total - conc - tied
    out        = 1 - (disc + 0.5*tied) / (disc + conc + tied + 1e-7)

Decomposition (host does O(N log N) rank/sort prep; the dense O(N^2)
pairwise counting runs on the NeuronCores):

 - Only event samples can be the i of a comparable pair.  Sort the E events
   by t-rank; then comp(i,j) = [i < c_j] is a PREFIX of the sorted order,
   with c_j = #{events with trank < u_j}, u_j = trank_j + 0.5*(1-d_j)
   (host searchsorted).  total = sum_j c_j needs no device work.
 - r is replaced by its dense rank over unique values, embedded as monotone
   fp16 bit patterns (1024 + 2*rank).view(f16); per-j thresholds use odd
   patterns (1024 + 2*rank + 1) so device compares are strict and tie-free:
     conc_j = #{i < c_j : remb_i > tgt_j},  tgt = emb(2*rrank_j + 1)
 - The tie band |fl32(r_i - r_j)| <= 1e-8 is a contiguous r-rank window
   [lo_j, hi_j] (host, exact IEEE-f32 semantics).  For float survival data
   the bands are ~1 rank wide, so `tied` is O(N)-sparse: counted exactly on
   host via combined-key searchsorted over (rank, t-position).  If bands are
   wide (heavily quantized r), fall back to counting all three sums on
   device (variant v1).
 - j's are sorted by c_j into 64 batches of 128 (partition dim).  Slot s
   holds batches [8s, 8s+8); core c takes batch 8s+c, so ONE SPMD NEFF
   serves all cores with per-slot static shapes F_s = min c, C_s = max c
   over the slot: counts over the prefix [0, F_s) are UNGATED reductions
   (tensor_scalar compare + accum on DVE in its 4x perf mode, or the
   activation-Sign trick on the Activation engine — both run concurrently);
   only the window [F_s, C_s) needs the [i < c_j] gate (iota-row compare +
   scalar_tensor_tensor on DVE).  Pool/GPSIMD cannot execute ALU ops under
   neuronxcc, but its SWDGE queue still serves as a third parallel DMA
   channel: the fp16 remb halves go out on the SP + Act HWDGE queues and
   iota+scalars on the Pool queue, all partition-broadcast (stride-0) so
   the host ships each row once.
"""

import numpy as np

N = 8192
NCORES = 8
P = 128
NBATCH = N // P            # 64 j-batches of 128
NSLOTS = NBATCH // NCORES  # 8 slots; slot s = batches [8s, 8s+8)

DVE, ACT = "dve", "act"

_CACHE = {}


# ----------------------------------------------------------------- host prep

def _tie_windows(rv):
    """For each unique sorted f32 value rv[k], the contiguous index window
    [lo[k], hi[k]] with |fl32(rv[p] - rv[k])| <= 1e-8 (exact IEEE f32
    semantics, matching the reference's rdiff).  Vectorized: f64 prefilter
    with slack, then exact f32 checks on boundary candidates."""
    m = len(rv)
    thr = np.float32(1e-8)
    rv64 = rv.astype(np.float64)
    pad = np.float64(1.1e-8)
    lo_ap = np.searchsorted(rv64, rv64 - pad, side="left")
    hi_ap = np.searchsorted(rv64, rv64 + pad, side="right") - 1

    def exact_in(k_idx, p_idx):
        p_idx = np.clip(p_idx, 0, m - 1)
        d = rv[p_idx] - rv[k_idx]          # exact f32 subtract
        return np.abs(d) <= thr

    ks = np.arange(m)
    lo = np.full(m, -1, dtype=np.int64)
    for off in range(-2, 3):
        cand = np.clip(lo_ap + off, 0, m - 1)
        ok = exact_in(ks, cand) & (lo < 0)
        lo[ok] = cand[ok]
    hi = np.full(m, -1, dtype=np.int64)
    for off in range(2, -3, -1):
        cand = np.clip(hi_ap + off, 0, m - 1)
        ok = exact_in(ks, cand) & (hi < 0)
        hi[ok] = cand[ok]
    bad = (lo < 0) | (lo > ks)
    lo[bad] = ks[bad]
    bad = (hi < 0) | (hi < ks)
    hi[bad] = ks[bad]
    return lo, hi


def _prep(event_indicator, event_time, estimate):
    d = np.asarray(event_indicator).reshape(-1).astype(bool)
    t = np.asarray(event_time, dtype=np.float32).reshape(-1)
    r = np.asarray(estimate, dtype=np.float32).reshape(-1)
    n = t.shape[0]
    assert n == N

    # t dense ranks (exp is strictly monotone, so raw times rank identically)
    tv = np.unique(t)
    trk = np.searchsorted(tv, t).astype(np.int64)
    assert len(tv) < 2040, "t ranks must stay fp16-exact"
    u = trk.astype(np.float32) + np.float32(0.5) * (~d).astype(np.float32)

    # events sorted by t-rank
    ev = np.nonzero(d)[0]
    E = len(ev)
    order = np.argsort(trk[ev], kind="stable")
    eidx = ev[order]
    ttr = trk[eidx]                       # sorted event t-ranks
    E_pad = max(((E + P - 1) // P) * P, P)

    # r dense ranks -> strict/odd monotone fp16 bit-pattern embedding
    rv = np.unique(r)
    m = len(rv)
    assert 1024 + 2 * m < 31744, "r rank embedding must stay normal fp16"
    rrk = np.searchsorted(rv, r)
    lo, hi = _tie_windows(rv)

    def emb(v):
        return (1024 + v).astype(np.uint16).view(np.float16)

    c = np.searchsorted(ttr, u, side="left").astype(np.int64)  # prefix cut
    total = float(c.sum())

    jorder = np.argsort(c, kind="stable")
    csort = c[jorder]

    slots = []
    for s in range(NSLOTS):
        cs = csort[s * NCORES * P:(s + 1) * NCORES * P]
        F = int(cs[0])
        C = int(cs[-1])
        slots.append({"F": F, "C": C, "W": C - F})

    return dict(d=d, u=u, trk=trk, eidx=eidx, ttr=ttr, E=E, E_pad=E_pad,
                rrk=rrk, lo=lo, hi=hi, emb=emb, c=c, total=total,
                jorder=jorder, slots=slots)


def _tied_on_host(pp):
    """tied = sum_j #{events i : i < c_j, rank_i in [lo_j, hi_j]} via
    combined-key searchsorted (exact).  Cost O(N log N + band width)."""
    rrk, lo, hi, c = pp["rrk"], pp["lo"], pp["hi"], pp["c"]
    eidx = pp["eidx"]
    BIG = np.int64(1 << 20)
    keys = np.sort(rrk[eidx].astype(np.int64) * BIG
                   + np.arange(len(eidx), dtype=np.int64))
    jr = rrk.astype(np.int64)
    jlo, jhi = lo[jr], hi[jr]

    def band_cnt(q, cj):                     # events rank==q with pos < c_j
        a = np.searchsorted(keys, q * BIG, side="left")
        b = np.searchsorted(keys, q * BIG + cj, side="left")
        return b - a

    tied = band_cnt(jr, c).sum()             # own-rank part, all j
    wide = np.nonzero(jhi > jlo)[0]
    for j in wide:
        for q in range(int(jlo[j]), int(jhi[j]) + 1):
            if q == jr[j]:
                continue
            tied += band_cnt(np.int64(q), int(c[j]))
    return float(tied)


# ----------------------------------------------------------- engine planning

def _seg_cost(eng, L):
    if eng == DVE:
        return 0.26 * L + 90
    return 0.833 * L + 430          # ACT


def _plan_v2(slots, E_pad):
    """DVE: all gated windows (comp + stt) + a share of full prefixes;
    ACT: the remaining full prefixes (Sign trick).  Full prefixes are
    pre-split at the stage boundary MID (end of the second DMA chunk) so
    stage-A work can start before the whole remb row lands."""
    MID = 2 * (E_pad // 4)
    loads = {DVE: 850.0, ACT: 0.0}
    items = []
    for s in range(NSLOTS):
        W = slots[s]["W"]
        if W == 0:
            continue
        loads[DVE] += 1.34 * W + 150
        items.append({"kind": "gated", "slot": s, "eng": DVE, "counts": [0],
                      "ext": slots[s]["C"]})

    pieces = []
    for s in range(NSLOTS):
        F = slots[s]["F"]
        if F == 0:
            continue
        pieces.append((s, 0, 0, min(F, MID)))
        if F > MID:
            pieces.append((s, 0, MID, F - MID))
    segs = _assign_full(pieces, loads, MID)
    items.extend(segs)
    return _assign_cols(items)


def _plan_v1(slots, E_pad):
    """Three counts on device (fallback for wide tie bands)."""
    MID = 2 * (E_pad // 4)
    loads = {DVE: 850.0, ACT: 0.0}
    items = []
    for s in range(NSLOTS):
        W = slots[s]["W"]
        if W == 0:
            continue
        loads[DVE] += 0.26 * W + 90 + 3 * (1.04 * W + 90)
        items.append({"kind": "gated", "slot": s, "eng": DVE,
                      "counts": [0, 1, 2], "ext": slots[s]["C"]})
    pieces = []
    for s in range(NSLOTS):
        F = slots[s]["F"]
        if F == 0:
            continue
        for k in range(3):
            pieces.append((s, k, 0, min(F, MID)))
            if F > MID:
                pieces.append((s, k, MID, F - MID))
    segs = _assign_full(pieces, loads, MID)
    items.extend(segs)
    return _assign_cols(items)


def _assign_full(pieces, loads, MID):
    """Greedy engine assignment of full-count pieces (slot, count, st, L),
    then one split-refinement toward balance."""
    segs = []
    for (s, k, st, L) in sorted(pieces, key=lambda p: -p[3]):
        eng = min((DVE, ACT), key=lambda e: loads[e] + _seg_cost(e, L))
        loads[eng] += _seg_cost(eng, L)
        segs.append({"kind": "full", "slot": s, "count": k, "eng": eng,
                     "seg": (st, L), "ext": st + L})
    for _ in range(16):
        hi_e = max(loads, key=loads.get)
        lo_e = min(loads, key=loads.get)
        cand = [g for g in segs if g["eng"] == hi_e and g["seg"][1] >= 512]
        if not cand or loads[hi_e] - loads[lo_e] < 300:
            break
        g = max(cand, key=lambda g: g["seg"][1])
        st, L = g["seg"]
        r_hi = 0.26 if hi_e == DVE else 0.833
        r_lo = 0.26 if lo_e == DVE else 0.833
        ovh = 90 if lo_e == DVE else 430
        gap = loads[hi_e] - loads[lo_e]
        move = int(min(L - 256, max(0, (gap - ovh) / (r_hi + r_lo))))
        if move < 256:
            break
        g["seg"] = (st, L - move)
        loads[hi_e] -= r_hi * move
        loads[lo_e] += r_lo * move + ovh
        segs.append({"kind": "full", "slot": g["slot"], "count": g["count"],
                     "eng": lo_e, "seg": (st + L - move, move),
                     "ext": st + L})
    return segs


def _assign_cols(items):
    cur = 0
    for it in items:
        it["col"] = cur
        cur += len(it["counts"]) if it["kind"] == "gated" else 1
    return items, max(cur, 1)


# ------------------------------------------------------------------ nc build

def _build_nc(slots, E_pad, Wmax, items, ncols, nsc):
    """Shared builder.  nsc = scalar columns per slot (2 for v2, 4 for v1).
    DMA: two HWDGE queues only (SP + Act); scalars and the iota row go
    first (tiny), then the remb row in four stride-0 partition-broadcast
    chunks alternating between the queues.  Stage-A items need only
    remb[0:MID]; stage-B the full row."""
    import concourse.bass as bass
    from concourse import mybir

    dt = mybir.dt
    Alu = mybir.AluOpType
    Af = mybir.ActivationFunctionType

    nc = bass.Bass()
    xrows = nc.declare_dram_parameter("xrows", [1, E_pad], dt.float16,
                                      isOutput=False)
    xsc = nc.declare_dram_parameter("xsc", [P, nsc * NSLOTS], dt.float32,
                                    isOutput=False)
    out = nc.declare_dram_parameter("out", [P, ncols], dt.float32,
                                    isOutput=True)
    CH = E_pad // 4
    MID = 2 * CH

    iota_dt = dt.float16 if Wmax <= 2040 else dt.float32
    with (
        nc.sbuf_tensor([P, E_pad], dt.float16) as remb,
        nc.sbuf_tensor([P, Wmax], iota_dt) as iota,
        nc.sbuf_tensor([P, nsc * NSLOTS], dt.float32) as sc,
        nc.sbuf_tensor([P, ncols], dt.float32) as acc,
        nc.sbuf_tensor([P, E_pad], dt.float16) as dead_v,
        nc.sbuf_tensor([P, E_pad], dt.float16) as dead_a,
        nc.sbuf_tensor([P, Wmax], dt.float16) as comp_v,
        nc.semaphore() as d1,
        nc.semaphore() as d2,
        nc.semaphore() as psem,
        nc.semaphore() as vsem,
        nc.Block() as block,
    ):
        def cutcol(s):
            return sc[:, nsc * s + 0:nsc * s + 1]

        def thrcol(s, k):
            return sc[:, nsc * s + 1 + k:nsc * s + 2 + k]

        def rchunk(lo_, hi_):
            return (remb[:, lo_:hi_],
                    xrows[0:1, lo_:hi_].partition_broadcast(P))

        @block.sync
        def _(sp):
            sp.dma_start(sc[:], xsc[:]).then_inc(d1, 16)
            sp.dma_start(*rchunk(0, CH)).then_inc(d1, 16)
            sp.dma_start(*rchunk(CH, MID)).then_inc(d1, 16)
            sp.wait_ge(vsem, 2)
            sp.dma_start(out[:], acc[:]).then_inc(d1, 16)

        @block.gpsimd
        def _(g):
            g.dma_start(*rchunk(MID, 3 * CH)).then_inc(d2, 16)
            g.dma_start(*rchunk(3 * CH, E_pad)).then_inc(d2, 16)

        # wait ladder: SP queue (d1): scalars(16), chunk1(32), chunk2(48);
        # Pool queue (d2): chunk3(16), chunk4(32).  The iota row is built
        # on the DVE during the startup window (memset + scan), so gated
        # items need no extra wait.
        def level_of(it):
            ext = it["ext"]
            if ext <= CH:
                return (32, 0)
            if ext <= MID:
                return (48, 0)
            if ext <= 3 * CH:
                return (48, 16)
            return (48, 32)

        def emit_staged(eng_api, eng_name):
            last = None
            done = (0, 0)
            order = sorted((it for it in items if it["eng"] == eng_name),
                           key=level_of)
            for it in order:
                lv = level_of(it)
                if lv[0] > done[0]:
                    eng_api.wait_ge(d1, lv[0])
                if lv[1] > done[1]:
                    eng_api.wait_ge(d2, lv[1])
                done = (max(lv[0], done[0]), max(lv[1], done[1]))
                s = it["slot"]
                if it["kind"] == "gated":
                    F, W = slots[s]["F"], slots[s]["W"]
                    eng_api.tensor_scalar(
                        comp_v[:, 0:W], iota[:, 0:W], cutcol(s),
                        None, Alu.is_lt)
                    for k in it["counts"]:
                        last = eng_api.scalar_tensor_tensor(
                            dead_v[:, 0:W], remb[:, F:F + W],
                            thrcol(s, k), comp_v[:, 0:W],
                            op0=Alu.is_gt, op1=Alu.mult,
                            accum_out=acc[:, it["col"] + k:
                                          it["col"] + k + 1])
                else:
                    st, L = it["seg"]
                    k = it["count"]
                    if eng_name == DVE:
                        last = eng_api.tensor_scalar(
                            dead_v[:, 0:L], remb[:, st:st + L],
                            thrcol(s, k), None, Alu.is_gt,
                            op1=Alu.add,
                            accum_out=acc[:, it["col"]:it["col"] + 1])
                    else:
                        last = eng_api.activation(
                            dead_a[:, 0:L], remb[:, st:st + L], Af.Sign,
                            bias=thrcol(s, k), scale=-1.0,
                            accum_out=acc[:, it["col"]:it["col"] + 1])
            return last

        @block.scalar
        def _(a):
            # warm the Sign activation table while the DMAs are in flight
            a.wait_ge(psem, 1)
            a.activation(dead_a[:, 0:1], dead_a[:, 0:1], Af.Sign,
                         bias=0.0, scale=1.0)
            last = emit_staged(a, ACT)
            if last is None:
                a.wait_ge(d1, 32)
                a.wait_ge(d2, 32)
                last = a.activation(dead_a[:, 0:1], remb[:, 0:1], Af.Sign,
                                    bias=0.0, scale=1.0)
            last.then_inc(vsem, 1)

        @block.vector
        def _(v):
            v.memset(dead_a[:, 0:1], 0.0).then_inc(psem, 1)
            v.memset(comp_v[:], 1.0)
            v.tensor_tensor_scan(iota[:], comp_v[:], comp_v[:], -1.0,
                                 Alu.add, Alu.bypass)
            last = emit_staged(v, DVE)
            if last is None:
                v.wait_ge(d1, 32)
                v.wait_ge(d2, 32)
                last = v.memset(dead_v[:, 0:1], 0.0)
            last.then_inc(vsem, 1)

    return nc


# --------------------------------------------------------------- input maps

def _inputs(pp, Wmax, nsc):
    emb, rrk, eidx, c, jorder = (pp["emb"], pp["rrk"], pp["eidx"], pp["c"],
                                 pp["jorder"])
    E, E_pad, slots = pp["E"], pp["E_pad"], pp["slots"]
    remb_row = np.zeros(E_pad, dtype=np.float16)
    remb_row[:E] = emb(2 * rrk[eidx])
    rows = remb_row[None, :]
    tgt = emb(2 * rrk + 1).astype(np.float32)
    if nsc == 4:
        tlo = emb(2 * pp["lo"][rrk] - 1).astype(np.float32)
        thi = emb(2 * pp["hi"][rrk] + 1).astype(np.float32)

    sc = np.zeros((NCORES, P, nsc * NSLOTS), dtype=np.float32)
    for s in range(NSLOTS):
        F = slots[s]["F"]
        for cr in range(NCORES):
            b = s * NCORES + cr
            jj = jorder[b * P:(b + 1) * P]
            sc[cr, :, nsc * s + 0] = (c[jj] - F).astype(np.float32)
            sc[cr, :, nsc * s + 1] = tgt[jj]
            if nsc == 4:
                sc[cr, :, nsc * s + 2] = tlo[jj]
                sc[cr, :, nsc * s + 3] = thi[jj]
    return [{"xrows": np.ascontiguousarray(rows),
             "xsc": np.ascontiguousarray(sc[cr])} for cr in range(NCORES)]


# ----------------------------------------------------------------- finishing

def _counts(results, items, nsums):
    acc = [np.float64(0.0)] * nsums
    for res in results:
        o = res["out"].astype(np.float64)
        for it in items:
            if it["kind"] == "gated":
                for k in it["counts"]:
                    acc[k] += o[:, it["col"] + k].sum()
            elif it["eng"] == ACT:
                L = it["seg"][1]
                acc[it.get("count", 0)] += (P * L - o[:, it["col"]].sum()) / 2.0
            else:
                acc[it.get("count", 0)] += o[:, it["col"]].sum()
    return acc


def _formula(total, conc, tied):
    disc = total - conc - tied
    loss = (disc + 0.5 * tied) / (disc + conc + tied + 1e-7)
    return np.asarray(1.0 - loss, dtype=np.float32)


# --------------------------------------------------------------- entry point

def kernel(event_indicator, event_time, estimate):
    from concourse.bass_utils import run_bass_kernel_spmd

    pp = _prep(event_indicator, event_time, estimate)
    slots, E_pad = pp["slots"], pp["E_pad"]
    total = pp["total"]
    if total == 0.0:
        return np.float32(1.0)

    jr = pp["rrk"]
    band = (pp["hi"][jr] - pp["lo"][jr]).sum()
    Wmax = max(max(s["W"] for s in slots), 1)
    use_v2 = band <= 8 * N

    if use_v2:
        tied = _tied_on_host(pp)
        items, ncols = _plan_v2(slots, E_pad)
        nsc = 2
    else:
        tied = None
        items, ncols = _plan_v1(slots, E_pad)
        nsc = 4

    key = (use_v2, E_pad, Wmax, ncols,
           tuple((s["F"], s["C"]) for s in slots),
           tuple((it["kind"], it["slot"], it.get("count", 0), it["eng"],
                  it.get("seg")) for it in items))
    if _CACHE.get("key") != key:
        _CACHE["nc"] = _build_nc(slots, E_pad, Wmax, items, ncols, nsc)
        _CACHE["key"] = key
    in_maps = _inputs(pp, Wmax, nsc)
    out = run_bass_kernel_spmd(_CACHE["nc"], in_maps,
                               core_ids=list(range(NCORES)))
    if use_v2:
        (conc,) = _counts(out.results, items, 1)
        return _formula(total, conc, tied)
    conc, cntA, cntB = _counts(out.results, items, 3)
    return _formula(total, conc, cntA - cntB)
